# revision 13
# baseline (speedup 1.0000x reference)
"""Trainium2 Bass kernel for a ResNet bottleneck block (training-mode BN).

Computes, for x of shape (64, 1024, 14, 14):
    y1 = relu(bn(conv1x1(x, w1)))        # 1024 -> 256
    y2 = relu(bn(conv3x3(y1, w2)))       # 256 -> 256, pad 1
    z3 = bn(conv1x1(y2, w3))             # 256 -> 1024
    out = relu(x + softplus(residual_scale) * z3)

BN is training-mode: per-channel mean/var over (N, H, W) of the full batch.

Sharding: data-parallel over the batch dim, 8 images per NeuronCore. Exact
global BN statistics: each core computes local per-channel (count, mean,
n*var) triples with the DVE's one-pass bn_stats/bn_aggr instructions,
AllGathers the triples (tiny, latency-bound) and combines them with a
post-gather bn_aggr. A dummy AllGather is issued at the very top of the
program so the runtime's first-collective barrier (which absorbs cross-core
launch skew and comm setup, ~50us) overlaps with the input DMA and conv1
instead of sitting on the critical path before the first BN exchange.

Conv bias is dropped: training-mode BN of (conv + b) is invariant to b.
softplus(residual_scale) is folded into gamma3/beta3 on the host.

All matmuls run in bf16 (fp32 PSUM accumulation). x is shipped once, in
bf16; the residual tail re-reads the same SBUF-resident bf16 copy. PSUM is
organized as 4 double-bank tiles so one ACT pass evicts two accumulation
groups. BN parameter math uses a single Rsqrt activation.
"""

import os
import numpy as np
import ml_dtypes
from contextlib import ExitStack

import concourse.bass as bass
import concourse.bacc as bacc
import concourse.mybir as mybir
import concourse.tile as tile
from concourse.bass_utils import run_bass_kernel_spmd

F32 = mybir.dt.float32
BF16 = mybir.dt.bfloat16
AX = mybir.AxisListType
ALU = mybir.AluOpType
ACTF = mybir.ActivationFunctionType

N_CORES = 8
N, CIN, H, W = 64, 1024, 14, 14
P = 256
COUT = 1024
NL = N // N_CORES          # images per core (8)
HW = H * W                 # 196
F = NL * HW                # free positions per core (1568)
FT = 4                     # free-dim tiles
FTS = F // FT              # 392 positions per tile (= 2 images)
IPT = NL // FT             # images per free tile (2)
CI_CH = CIN // 128         # 8
P_CH = P // 128            # 2
CO_CH = COUT // 128        # 8
EPS = 1e-5
PAD = 16                   # padded spatial stride (16x16 per image)
SG = 4                     # bn_stats groups per channel chunk (F = SG*392)


def _emit_exchange(nc, tc, ctx, name, st, n_ch, cc_mode):
    """AllGather local [128, 3, n_ch] (count, mean, n*var) triples and
    combine across cores with one bn_aggr per chunk.
    Returns mv: SBUF [128, n_ch, 2] fp32 of global (mean, var)."""
    dram = ctx.enter_context(tc.tile_pool(name=f"{name}_dram", bufs=1, space="DRAM"))
    sb = ctx.enter_context(tc.tile_pool(name=f"{name}_sb", bufs=1))

    # partition-major DRAM layout: the DMAs move contiguous 12*n_ch-byte
    # runs per partition instead of 4-byte packets.
    cc_in = dram.tile([128, 3 * n_ch], F32, name=f"{name}_in")
    cc_out = dram.tile([N_CORES, 128, 3 * n_ch], F32,
                       addr_space="Shared" if cc_mode == "ag" else "Local",
                       name=f"{name}_out")
    nc.sync.dma_start(cc_in[:], st.rearrange("p c t -> p (c t)"))
    if cc_mode == "ag":
        nc.gpsimd.collective_compute(
            "AllGather",
            ALU.bypass,
            replica_groups=[list(range(N_CORES))],
            ins=[cc_in.opt()],
            outs=[cc_out.opt()],
        )
    else:
        # debug: replicate local triples into every row (== BN with local
        # batch stats; close numerically for sanity checks)
        for r in range(N_CORES):
            nc.sync.dma_start(cc_out[r], cc_in[:])
    # [r, p, (s c)] -> SBUF [p, r, (s c)]
    gath = sb.tile([128, N_CORES, 3 * n_ch], F32, name=f"{name}_gath")
    nc.sync.dma_start(gath[:], cc_out.rearrange("r p x -> p r x"))
    mv = sb.tile([128, n_ch, 2], F32, name=f"{name}_mv")
    # regroup [p, r, (c t)] -> [p, c, r, t] so each chunk's 8 (count, mean,
    # var) triples are contiguous for bn_aggr.
    g2 = sb.tile([128, n_ch, N_CORES, 3], F32, name=f"{name}_g2")
    nc.vector.tensor_copy(g2[:], gath.rearrange("p r (c t) -> p c r t", c=n_ch))
    for c in range(n_ch):
        nc.vector.bn_aggr(mv[:, c], g2[:, c].rearrange("p r t -> p (r t)"))
    return mv


def _emit_bn_params(nc, tc, ctx, name, mv, gamma, beta, n_ch, epst):
    """Per-channel affine (a, b): bn(z) = a * z + b.
    a = gamma * rsqrt(var + eps), b = beta - a * mean."""
    sb = ctx.enter_context(tc.tile_pool(name=f"{name}_bn", bufs=1))
    std = sb.tile([128, n_ch], F32, name=f"{name}_std")
    nc.scalar.activation(std[:], mv[:, :, 1], ACTF.Sqrt, bias=epst[:, 0:1])
    ar = sb.tile([128, n_ch], F32, name=f"{name}_ar")
    nc.vector.reciprocal(ar[:], std[:])
    a = sb.tile([128, n_ch], F32, name=f"{name}_a")
    nc.vector.tensor_mul(a[:], ar[:], gamma[:])
    am = sb.tile([128, n_ch], F32, name=f"{name}_am")
    nc.vector.tensor_mul(am[:], a[:], mv[:, :, 0])
    b = sb.tile([128, n_ch], F32, name=f"{name}_b")
    nc.vector.scalar_tensor_tensor(
        b[:], am[:], -1.0, beta[:], op0=ALU.mult, op1=ALU.add
    )
    return a, b


def build():
    cc_mode = os.environ.get("KERNEL_CC_MODE", "ag")
    dummy_cc = os.environ.get("KERNEL_DUMMY_CC", "0") == "1" and cc_mode == "ag"
    nc = bacc.Bacc("TRN2", target_bir_lowering=False, debug=False,
                   num_devices=N_CORES)

    # ---- I/O -------------------------------------------------------------
    xb_d = nc.dram_tensor("xb16", [CI_CH, 128, F], BF16, kind="ExternalInput")
    w1_d = nc.dram_tensor("w1t", [CI_CH, 128, P], BF16, kind="ExternalInput")
    w2_d = nc.dram_tensor("w2t", [P_CH, 128, 9, P], BF16, kind="ExternalInput")
    w3_d = nc.dram_tensor("w3t", [P_CH, 128, COUT], BF16, kind="ExternalInput")
    gb1_d = nc.dram_tensor("gb1", [2, 128, P_CH], F32, kind="ExternalInput")
    gb2_d = nc.dram_tensor("gb2", [2, 128, P_CH], F32, kind="ExternalInput")
    gb3_d = nc.dram_tensor("gb3", [2, 128, CO_CH], F32, kind="ExternalInput")
    out_d = nc.dram_tensor("out", [NL, CIN, HW], F32, kind="ExternalOutput")

    with tile.TileContext(nc) as tc, ExitStack() as ctx:
        consts = ctx.enter_context(tc.tile_pool(name="consts", bufs=1))
        xpool = ctx.enter_context(tc.tile_pool(name="xpool", bufs=1))
        actp = ctx.enter_context(tc.tile_pool(name="actp", bufs=1))
        statp = ctx.enter_context(tc.tile_pool(name="statp", bufs=1))
        scrp = ctx.enter_context(tc.tile_pool(name="scrp", bufs=2))
        psum = ctx.enter_context(tc.tile_pool(name="psum", bufs=4, space="PSUM"))

        # ---- dummy collective: absorbs the runtime's first-collective
        # barrier (comm setup + cross-core launch skew) under conv1.
        if dummy_cc:
            dpool = ctx.enter_context(
                tc.tile_pool(name="dummy_dram", bufs=1, space="DRAM"))
            dum_in = dpool.tile([2, 128], F32, name="dummy_in")
            dum_out = dpool.tile([N_CORES, 2, 128], F32, addr_space="Shared",
                                 name="dummy_out")
            dscr = consts.tile([128, 2], F32, name="dummy_scr")
            nc.vector.memset(dscr[:], 0.0)
            nc.sync.dma_start(dum_in.rearrange("s p -> p s"), dscr[:])
            nc.gpsimd.collective_compute(
                "AllGather",
                ALU.bypass,
                replica_groups=[list(range(N_CORES))],
                ins=[dum_in.opt()],
                outs=[dum_out.opt()],
            )

        # ---- weights first on the sync queue (conv1's first matmul needs
        # w1 + all xb chunks), then the bulk x; single batched DMAs.
        w1sb_t = consts.tile([128, CI_CH, P], BF16, name="w1sb")
        nc.sync.dma_start(w1sb_t[:], w1_d.rearrange("c p k -> p c k"))
        w1sb = [w1sb_t[:, c] for c in range(CI_CH)]
        xb_t = xpool.tile([128, CI_CH, F], BF16, name="xb")
        nc.sync.dma_start(xb_t[:], xb_d.rearrange("c p f -> p c f"))
        xb = [xb_t[:, c] for c in range(CI_CH)]

        w2sb_t = consts.tile([128, P_CH, 9, P], BF16, name="w2sb")
        nc.scalar.dma_start(
            w2sb_t.rearrange("p c t k -> p c (t k)"),
            w2_d.rearrange("c p t k -> p c (t k)"))
        w2sb = [w2sb_t[:, c] for c in range(P_CH)]
        w3sb_t = consts.tile([128, P_CH, COUT], BF16, name="w3sb")
        nc.scalar.dma_start(w3sb_t[:], w3_d.rearrange("c p k -> p c k"))
        w3sb = [w3sb_t[:, c] for c in range(P_CH)]

        g1 = consts.tile([128, P_CH], F32, name="g1")
        be1 = consts.tile([128, P_CH], F32, name="be1")
        g2 = consts.tile([128, P_CH], F32, name="g2")
        be2 = consts.tile([128, P_CH], F32, name="be2")
        g3 = consts.tile([128, CO_CH], F32, name="g3")
        be3 = consts.tile([128, CO_CH], F32, name="be3")
        for t, d in ((g1, gb1_d), (g2, gb2_d), (g3, gb3_d)):
            nc.scalar.dma_start(t[:], d[0])
        for t, d in ((be1, gb1_d), (be2, gb2_d), (be3, gb3_d)):
            nc.scalar.dma_start(t[:], d[1])

        epst = consts.tile([128, 1], F32, name="epst")
        nc.vector.memset(epst[:], EPS)

        # padded bf16 activations for the 3x3 conv: [128, NL, 16, 16]
        y1p = [actp.tile([128, NL, PAD, PAD], BF16, name=f"y1p{c}")
               for c in range(P_CH)]
        for c in range(P_CH):
            nc.vector.memset(y1p[c][:], 0)

        z1 = [actp.tile([128, F], F32, name=f"z1_{c}") for c in range(P_CH)]
        z2 = [actp.tile([128, F], F32, name=f"z2_{c}") for c in range(P_CH)]
        y2 = [actp.tile([128, F], BF16, name=f"y2_{c}") for c in range(P_CH)]
        z3 = [actp.tile([128, F], BF16, name=f"z3_{c}") for c in range(CO_CH)]

        # local-stat staging: count planes pre-set to F
        bs1 = statp.tile([128, P_CH, SG, 6], F32, name="bs1")
        st1 = statp.tile([128, P_CH, 3], F32, name="st1")
        bs2 = statp.tile([128, P_CH, SG, 6], F32, name="bs2")
        st2 = statp.tile([128, P_CH, 3], F32, name="st2")
        bs3 = statp.tile([128, CO_CH, SG, 6], F32, name="bs3")
        st3 = statp.tile([128, CO_CH, 3], F32, name="st3")
        for st in (st1, st2, st3):
            nc.vector.memset(st[:, :, 0], 1.0)

        # ================= stage A: conv1 (1x1, 1024 -> 256) =============
        # NOTE: accumulation groups stay sequential per PSUM region
        # (group-outer, contraction-inner); interleaving groups across
        # banks hangs on hardware. PSUM tiles are double-bank [128,2,512]:
        # two groups accumulate into halves, one ACT evicts both.
        for co in range(P_CH):
            for fp in range(2):
                pt = psum.tile([128, 2, 512], F32, name="pt", tag="pt")
                for half in range(2):
                    ft = fp * 2 + half
                    for ci in range(CI_CH):
                        nc.tensor.matmul(
                            pt[:, half, :FTS],
                            w1sb[ci][:, co * 128:(co + 1) * 128],
                            xb[ci][:, ft * FTS:(ft + 1) * FTS],
                            start=(ci == 0),
                            stop=(ci == CI_CH - 1),
                        )
                nc.scalar.copy(
                    z1[co][:, fp * 2 * FTS:(fp + 1) * 2 * FTS]
                        .rearrange("p (a b) -> p a b", a=2),
                    pt[:, :, :FTS],
                )
            for g in range(SG):
                nc.vector.bn_stats(
                    bs1[:, co, g], z1[co][:, g * FTS:(g + 1) * FTS])
            nc.vector.bn_aggr(
                st1[:, co, 1:3], bs1[:, co].rearrange("p g s -> p (g s)"))
        mv1 = _emit_exchange(nc, tc, ctx, "bn1", st1, P_CH, cc_mode)
        a1, b1 = _emit_bn_params(nc, tc, ctx, "bn1", mv1, g1, be1, P_CH, epst)

        NH = NL // 2
        for h2 in range(2):
            for c in range(P_CH):
                nc.scalar.activation(
                    y1p[c][:, h2 * NH:(h2 + 1) * NH, 1:1 + H, 1:1 + W],
                    z1[c].rearrange("p (n h w) -> p n h w", n=NL, h=H, w=W)
                        [:, h2 * NH:(h2 + 1) * NH],
                    ACTF.Relu,
                    bias=b1[:, c:c + 1],
                    scale=a1[:, c:c + 1],
                )

        # ================= stage B: conv2 (3x3, 256 -> 256) ==============
        for co in range(P_CH):
            for fp in range(2):
                pt = psum.tile([128, 2, 512], F32, name="pt", tag="pt")
                for half in range(2):
                    ft = fp * 2 + half
                    for ci in range(P_CH):
                        for tap in range(9):
                            ky, kx = divmod(tap, 3)
                            nc.tensor.matmul(
                                pt[:, half, :FTS],
                                w2sb[ci][:, tap, co * 128:(co + 1) * 128],
                                y1p[ci][:, ft * IPT:(ft + 1) * IPT,
                                        ky:ky + H, kx:kx + W],
                                start=(ci == 0 and tap == 0),
                                stop=(ci == P_CH - 1 and tap == 8),
                            )
                nc.scalar.copy(
                    z2[co][:, fp * 2 * FTS:(fp + 1) * 2 * FTS]
                        .rearrange("p (a b) -> p a b", a=2),
                    pt[:, :, :FTS],
                )
            for g in range(SG):
                nc.vector.bn_stats(
                    bs2[:, co, g], z2[co][:, g * FTS:(g + 1) * FTS])
            nc.vector.bn_aggr(
                st2[:, co, 1:3], bs2[:, co].rearrange("p g s -> p (g s)"))
        mv2 = _emit_exchange(nc, tc, ctx, "bn2", st2, P_CH, cc_mode)
        a2, b2 = _emit_bn_params(nc, tc, ctx, "bn2", mv2, g2, be2, P_CH, epst)

        for h2 in range(2):
            for c in range(P_CH):
                nc.scalar.activation(
                    y2[c][:, h2 * 2 * FTS:(h2 + 1) * 2 * FTS],
                    z2[c][:, h2 * 2 * FTS:(h2 + 1) * 2 * FTS], ACTF.Relu,
                    bias=b2[:, c:c + 1], scale=a2[:, c:c + 1],
                )

        # ================= stage C: conv3 (1x1, 256 -> 1024) =============
        for co in range(CO_CH):
            for fp in range(2):
                pt = psum.tile([128, 2, 512], F32, name="pt", tag="pt")
                for half in range(2):
                    ft = fp * 2 + half
                    for ci in range(P_CH):
                        nc.tensor.matmul(
                            pt[:, half, :FTS],
                            w3sb[ci][:, co * 128:(co + 1) * 128],
                            y2[ci][:, ft * FTS:(ft + 1) * FTS],
                            start=(ci == 0),
                            stop=(ci == P_CH - 1),
                        )
                nc.scalar.copy(
                    z3[co][:, fp * 2 * FTS:(fp + 1) * 2 * FTS]
                        .rearrange("p (a b) -> p a b", a=2),
                    pt[:, :, :FTS],
                )
            for g in range(SG):
                nc.vector.bn_stats(
                    bs3[:, co, g], z3[co][:, g * FTS:(g + 1) * FTS])
            nc.vector.bn_aggr(
                st3[:, co, 1:3], bs3[:, co].rearrange("p g s -> p (g s)"))
        mv3 = _emit_exchange(nc, tc, ctx, "bn3", st3, CO_CH, cc_mode)
        a3, b3 = _emit_bn_params(nc, tc, ctx, "bn3", mv3, g3, be3, CO_CH, epst)

        # tail: out = relu((a3*z3 + x) + b3); t in bf16 (2x DVE mode), the
        # relu pass converts to fp32 for the output DMA.
        # tail per chunk: t = a3*z3 (TS, 4x bf16) ; t += x (TT, 2x bf16) ;
        # out = relu(t + b3) fp32 — ACT for six chunks, DVE add+max for two.
        outf = [actp.tile([128, F], F32, name=f"outf{c}") for c in range(CO_CH)]
        tts = [scrp.tile([128, F], BF16, name=f"t{c}", tag=f"t{c % 4}")
               for c in range(CO_CH)]
        for co in range(CO_CH):
            t = tts[co]
            nc.vector.tensor_scalar_mul(t[:], z3[co][:], a3[:, co:co + 1])
            nc.vector.tensor_add(t[:], t[:], xb[co][:])
            if co < 6:
                nc.scalar.activation(
                    outf[co][:], t[:], ACTF.Relu, bias=b3[:, co:co + 1],
                )
            else:
                nc.vector.tensor_scalar(
                    outf[co][:], t[:], b3[:, co:co + 1], 0.0,
                    op0=ALU.add, op1=ALU.max,
                )
            deng = nc.sync if co % 2 == 0 else nc.scalar
            deng.dma_start(
                out_d[:, co * 128:(co + 1) * 128, :].rearrange("n p f -> p n f"),
                outf[co][:],
            )
    nc.compile()
    return nc


_NC_CACHE = None


def _get_nc():
    global _NC_CACHE
    if _NC_CACHE is None:
        _NC_CACHE = build()
    return _NC_CACHE


def _prep_host(w1, w2, w3, g1, be1, g2, be2, g3, be3, residual_scale):
    bf = ml_dtypes.bfloat16
    # conv weights, pre-transposed to [ci, ...] layouts for lhsT
    w1t = np.ascontiguousarray(
        w1.reshape(P, CIN).T.astype(bf)).reshape(CI_CH, 128, P)
    w2t = np.ascontiguousarray(
        w2.transpose(1, 2, 3, 0).astype(bf)).reshape(P_CH, 128, 9, P)
    w3t = np.ascontiguousarray(
        w3.reshape(COUT, P).T.astype(bf)).reshape(P_CH, 128, COUT)
    s = np.float32(np.log1p(np.exp(np.float64(residual_scale[0]))))
    gb1 = np.ascontiguousarray(np.stack([g1, be1]).astype(np.float32)
                               .reshape(2, P_CH, 128).transpose(0, 2, 1))
    gb2 = np.ascontiguousarray(np.stack([g2, be2]).astype(np.float32)
                               .reshape(2, P_CH, 128).transpose(0, 2, 1))
    gb3 = np.ascontiguousarray((np.stack([g3, be3]) * s).astype(np.float32)
                               .reshape(2, CO_CH, 128).transpose(0, 2, 1))
    return w1t, w2t, w3t, gb1, gb2, gb3


def prepare_in_maps(inputs):
    x = np.asarray(inputs["x"], dtype=np.float32)
    w1t, w2t, w3t, gb1, gb2, gb3 = _prep_host(
        np.asarray(inputs["w1"], np.float32), np.asarray(inputs["w2"], np.float32),
        np.asarray(inputs["w3"], np.float32), np.asarray(inputs["g1"], np.float32),
        np.asarray(inputs["be1"], np.float32), np.asarray(inputs["g2"], np.float32),
        np.asarray(inputs["be2"], np.float32), np.asarray(inputs["g3"], np.float32),
        np.asarray(inputs["be3"], np.float32),
        np.asarray(inputs["residual_scale"], np.float32),
    )
    in_maps = []
    for c in range(N_CORES):
        shard = x[c * NL:(c + 1) * NL].reshape(NL, CIN, HW)
        xb16 = np.ascontiguousarray(
            shard.transpose(1, 0, 2).astype(ml_dtypes.bfloat16)
        ).reshape(CI_CH, 128, F)
        in_maps.append({
            "xb16": xb16, "w1t": w1t, "w2t": w2t, "w3t": w3t,
            "gb1": gb1, "gb2": gb2, "gb3": gb3,
        })
    return in_maps


def kernel(**inputs):
    in_maps = prepare_in_maps(inputs)
    nc = _get_nc()
    trace = bool(int(os.environ.get("KERNEL_PROFILE", "0")))
    try:
        res = run_bass_kernel_spmd(nc, in_maps, list(range(N_CORES)), trace=trace)
    except ModuleNotFoundError:
        # axon NTFF profile hook unavailable in this container
        res = run_bass_kernel_spmd(nc, in_maps, list(range(N_CORES)), trace=False)
    if trace:
        kernel.last_exec_time_ns = getattr(res, "exec_time_ns", None)
        kernel.last_profile = res
    out = np.concatenate([res.results[c]["out"] for c in range(N_CORES)], axis=0)
    return out.reshape(N, CIN, H, W)


# revision 15
# speedup vs baseline: 1.4152x; 1.4152x over previous
"""Trainium2 Bass kernel for a ResNet bottleneck block (training-mode BN).

Computes, for x of shape (64, 1024, 14, 14):
    y1 = relu(bn(conv1x1(x, w1)))        # 1024 -> 256
    y2 = relu(bn(conv3x3(y1, w2)))       # 256 -> 256, pad 1
    z3 = bn(conv1x1(y2, w3))             # 256 -> 1024
    out = relu(x + softplus(residual_scale) * z3)

BN is training-mode: per-channel mean/var over (N, H, W) of the full batch.

Sharding: data-parallel over the batch dim, 8 images per NeuronCore. Exact
global BN statistics: each core computes local per-channel (count, mean,
var) triples with the DVE's one-pass bn_stats/bn_aggr instructions,
AllGathers the triples (tiny, latency-bound, partition-major DRAM layout so
the DMAs move contiguous runs) and combines them with a post-gather
bn_aggr (exact for equal group counts).

Conv bias is dropped: training-mode BN of (conv + b) is invariant to b.
softplus(residual_scale) is folded into gamma3/beta3 on the host.

All matmuls run in bf16 (fp32 PSUM accumulation). x is shipped once, in
bf16; the residual tail re-reads the same SBUF-resident bf16 copy. PSUM is
organized as 4 double-bank tiles so one ACT pass evicts two accumulation
groups. The tail splits relu work between ACT and DVE and computes
a3*z3 + x as tensor_scalar + tensor_tensor (fast DVE perf modes).
"""

import os
import numpy as np
import ml_dtypes
from contextlib import ExitStack

import concourse.bass as bass
import concourse.bacc as bacc
import concourse.mybir as mybir
import concourse.tile as tile
from concourse.bass_utils import run_bass_kernel_spmd

F32 = mybir.dt.float32
BF16 = mybir.dt.bfloat16
AX = mybir.AxisListType
ALU = mybir.AluOpType
ACTF = mybir.ActivationFunctionType

N_CORES = 8
N, CIN, H, W = 64, 1024, 14, 14
P = 256
COUT = 1024
NL = N // N_CORES          # images per core (8)
HW = H * W                 # 196
F = NL * HW                # free positions per core (1568)
FT = 4                     # free-dim tiles
FTS = F // FT              # 392 positions per tile (= 2 images)
IPT = NL // FT             # images per free tile (2)
CI_CH = CIN // 128         # 8
P_CH = P // 128            # 2
CO_CH = COUT // 128        # 8
EPS = 1e-5
PAD = 16                   # padded spatial stride (16x16 per image)
SG = 4                     # bn_stats groups per channel chunk (F = SG*392)


def _emit_exchange(nc, tc, ctx, name, st, n_ch, cc_mode):
    """AllGather local [128, 3, n_ch] (count, mean, n*var) triples and
    combine across cores with one bn_aggr per chunk.
    Returns mv: SBUF [128, n_ch, 2] fp32 of global (mean, var)."""
    dram = ctx.enter_context(tc.tile_pool(name=f"{name}_dram", bufs=1, space="DRAM"))
    sb = ctx.enter_context(tc.tile_pool(name=f"{name}_sb", bufs=1))

    # partition-major DRAM layout: the DMAs move contiguous 12*n_ch-byte
    # runs per partition instead of 4-byte packets.
    cc_in = dram.tile([128, 3 * n_ch], F32, name=f"{name}_in")
    cc_out = dram.tile([N_CORES, 128, 3 * n_ch], F32,
                       addr_space="Shared" if cc_mode == "ag" else "Local",
                       name=f"{name}_out")
    nc.sync.dma_start(cc_in[:], st.rearrange("p c t -> p (c t)"))
    if cc_mode == "ag":
        nc.gpsimd.collective_compute(
            "AllGather",
            ALU.bypass,
            replica_groups=[list(range(N_CORES))],
            ins=[cc_in.opt()],
            outs=[cc_out.opt()],
        )
    else:
        # debug: replicate local triples into every row (== BN with local
        # batch stats; close numerically for sanity checks)
        for r in range(N_CORES):
            nc.sync.dma_start(cc_out[r], cc_in[:])
    # [r, p, (s c)] -> SBUF [p, r, (s c)]
    gath = sb.tile([128, N_CORES, 3 * n_ch], F32, name=f"{name}_gath")
    nc.sync.dma_start(gath[:], cc_out.rearrange("r p x -> p r x"))
    mv = sb.tile([128, n_ch, 2], F32, name=f"{name}_mv")
    # regroup [p, r, (c t)] -> [p, c, r, t] so each chunk's 8 (count, mean,
    # var) triples are contiguous for bn_aggr.
    g2 = sb.tile([128, n_ch, N_CORES, 3], F32, name=f"{name}_g2")
    nc.vector.tensor_copy(g2[:], gath.rearrange("p r (c t) -> p c r t", c=n_ch))
    for c in range(n_ch):
        nc.vector.bn_aggr(mv[:, c], g2[:, c].rearrange("p r t -> p (r t)"))
    return mv


def _emit_bn_params(nc, tc, ctx, name, mv, gamma, beta, n_ch, epst):
    """Per-channel affine (a, b): bn(z) = a * z + b.
    a = gamma * rsqrt(var + eps), b = beta - a * mean."""
    sb = ctx.enter_context(tc.tile_pool(name=f"{name}_bn", bufs=1))
    std = sb.tile([128, n_ch], F32, name=f"{name}_std")
    nc.scalar.activation(std[:], mv[:, :, 1], ACTF.Sqrt, bias=epst[:, 0:1])
    ar = sb.tile([128, n_ch], F32, name=f"{name}_ar")
    nc.vector.reciprocal(ar[:], std[:])
    a = sb.tile([128, n_ch], F32, name=f"{name}_a")
    nc.vector.tensor_mul(a[:], ar[:], gamma[:])
    am = sb.tile([128, n_ch], F32, name=f"{name}_am")
    nc.vector.tensor_mul(am[:], a[:], mv[:, :, 0])
    b = sb.tile([128, n_ch], F32, name=f"{name}_b")
    nc.vector.scalar_tensor_tensor(
        b[:], am[:], -1.0, beta[:], op0=ALU.mult, op1=ALU.add
    )
    return a, b


def _emit_local_stats(nc, name, statp, bs, st, agg, n_ch):
    """bs [128, n_ch, SG, 6] bn_stats outputs -> st [128, n_ch, 3] local
    (1, mean, var) triples, aggregated straight into st[:, c, 1:3].
    Count 1 with the local var in the "count*var" slot is exact for the
    post-gather bn_aggr because all groups have equal true counts."""
    for c in range(n_ch):
        nc.vector.bn_aggr(st[:, c, 1:3], bs[:, c].rearrange("p g s -> p (g s)"))


def build():
    cc_mode = os.environ.get("KERNEL_CC_MODE", "ag")
    dummy_cc = os.environ.get("KERNEL_DUMMY_CC", "0") == "1" and cc_mode == "ag"
    nc = bacc.Bacc("TRN2", target_bir_lowering=False, debug=False,
                   num_devices=N_CORES)

    # ---- I/O -------------------------------------------------------------
    xb_d = nc.dram_tensor("xb16", [CI_CH, 128, F], BF16, kind="ExternalInput")
    w1_d = nc.dram_tensor("w1t", [CI_CH, 128, P], BF16, kind="ExternalInput")
    w2_d = nc.dram_tensor("w2t", [P_CH, 128, 9, P], BF16, kind="ExternalInput")
    w3_d = nc.dram_tensor("w3t", [P_CH, 128, COUT], BF16, kind="ExternalInput")
    gb1_d = nc.dram_tensor("gb1", [2, 128, P_CH], F32, kind="ExternalInput")
    gb2_d = nc.dram_tensor("gb2", [2, 128, P_CH], F32, kind="ExternalInput")
    gb3_d = nc.dram_tensor("gb3", [2, 128, CO_CH], F32, kind="ExternalInput")
    out_d = nc.dram_tensor("out", [NL, CIN, HW], F32, kind="ExternalOutput")

    with tile.TileContext(nc) as tc, ExitStack() as ctx:
        consts = ctx.enter_context(tc.tile_pool(name="consts", bufs=1))
        xpool = ctx.enter_context(tc.tile_pool(name="xpool", bufs=1))
        actp = ctx.enter_context(tc.tile_pool(name="actp", bufs=1))
        statp = ctx.enter_context(tc.tile_pool(name="statp", bufs=1))
        scrp = ctx.enter_context(tc.tile_pool(name="scrp", bufs=2))
        psum = ctx.enter_context(tc.tile_pool(name="psum", bufs=4, space="PSUM"))

        # ---- dummy collective: absorbs the runtime's first-collective
        # barrier (comm setup + cross-core launch skew) under conv1.
        if dummy_cc:
            dpool = ctx.enter_context(
                tc.tile_pool(name="dummy_dram", bufs=1, space="DRAM"))
            dum_in = dpool.tile([2, 128], F32, name="dummy_in")
            dum_out = dpool.tile([N_CORES, 2, 128], F32, addr_space="Shared",
                                 name="dummy_out")
            dscr = consts.tile([128, 2], F32, name="dummy_scr")
            nc.vector.memset(dscr[:], 0.0)
            nc.sync.dma_start(dum_in.rearrange("s p -> p s"), dscr[:])
            nc.gpsimd.collective_compute(
                "AllGather",
                ALU.bypass,
                replica_groups=[list(range(N_CORES))],
                ins=[dum_in.opt()],
                outs=[dum_out.opt()],
            )

        # ---- weights first on the sync queue (conv1's first matmul needs
        # w1 + all xb chunks), then the bulk x; single batched DMAs.
        w1sb_t = consts.tile([128, CI_CH, P], BF16, name="w1sb")
        nc.sync.dma_start(w1sb_t[:], w1_d.rearrange("c p k -> p c k"))
        w1sb = [w1sb_t[:, c] for c in range(CI_CH)]
        xb_t = xpool.tile([128, CI_CH, F], BF16, name="xb")
        nc.sync.dma_start(xb_t[:], xb_d.rearrange("c p f -> p c f"))
        xb = [xb_t[:, c] for c in range(CI_CH)]

        w2sb_t = consts.tile([128, P_CH, 9, P], BF16, name="w2sb")
        nc.scalar.dma_start(
            w2sb_t.rearrange("p c t k -> p c (t k)"),
            w2_d.rearrange("c p t k -> p c (t k)"))
        w2sb = [w2sb_t[:, c] for c in range(P_CH)]
        w3sb_t = consts.tile([128, P_CH, COUT], BF16, name="w3sb")
        nc.scalar.dma_start(w3sb_t[:], w3_d.rearrange("c p k -> p c k"))
        w3sb = [w3sb_t[:, c] for c in range(P_CH)]

        g1 = consts.tile([128, P_CH], F32, name="g1")
        be1 = consts.tile([128, P_CH], F32, name="be1")
        g2 = consts.tile([128, P_CH], F32, name="g2")
        be2 = consts.tile([128, P_CH], F32, name="be2")
        g3 = consts.tile([128, CO_CH], F32, name="g3")
        be3 = consts.tile([128, CO_CH], F32, name="be3")
        for t, d in ((g1, gb1_d), (g2, gb2_d), (g3, gb3_d)):
            nc.scalar.dma_start(t[:], d[0])
        for t, d in ((be1, gb1_d), (be2, gb2_d), (be3, gb3_d)):
            nc.scalar.dma_start(t[:], d[1])

        epst = consts.tile([128, 1], F32, name="epst")
        nc.vector.memset(epst[:], EPS)

        # padded bf16 activations for the 3x3 conv: [128, NL, 16, 16]
        y1p = [actp.tile([128, NL, PAD, PAD], BF16, name=f"y1p{c}")
               for c in range(P_CH)]
        for c in range(P_CH):
            nc.vector.memset(y1p[c][:], 0)

        z1 = [actp.tile([128, F], F32, name=f"z1_{c}") for c in range(P_CH)]
        z2 = [actp.tile([128, F], F32, name=f"z2_{c}") for c in range(P_CH)]
        y2 = [actp.tile([128, F], BF16, name=f"y2_{c}") for c in range(P_CH)]
        z3 = [actp.tile([128, F], BF16, name=f"z3_{c}") for c in range(CO_CH)]

        # local-stat staging: count planes pre-set to F
        bs1 = statp.tile([128, P_CH, SG, 6], F32, name="bs1")
        st1 = statp.tile([128, P_CH, 3], F32, name="st1")
        bs2 = statp.tile([128, P_CH, SG, 6], F32, name="bs2")
        st2 = statp.tile([128, P_CH, 3], F32, name="st2")
        bs3 = statp.tile([128, CO_CH, SG, 6], F32, name="bs3")
        st3 = statp.tile([128, CO_CH, 3], F32, name="st3")
        agg1 = agg2 = agg3 = None
        for st in (st1, st2, st3):
            nc.vector.memset(st[:, :, 0], 1.0)

        # ================= stage A: conv1 (1x1, 1024 -> 256) =============
        # NOTE: accumulation groups stay sequential per PSUM region
        # (group-outer, contraction-inner); interleaving groups across
        # banks hangs on hardware. PSUM tiles are double-bank [128,2,512]:
        # two groups accumulate into halves, one ACT evicts both.
        for co in range(P_CH):
            for fp in range(2):
                pt = psum.tile([128, 2, 512], F32, name="pt", tag="pt")
                for half in range(2):
                    ft = fp * 2 + half
                    for ci in range(CI_CH):
                        nc.tensor.matmul(
                            pt[:, half, :FTS],
                            w1sb[ci][:, co * 128:(co + 1) * 128],
                            xb[ci][:, ft * FTS:(ft + 1) * FTS],
                            start=(ci == 0),
                            stop=(ci == CI_CH - 1),
                        )
                nc.scalar.copy(
                    z1[co][:, fp * 2 * FTS:(fp + 1) * 2 * FTS]
                        .rearrange("p (a b) -> p a b", a=2),
                    pt[:, :, :FTS],
                )
            for g in range(SG):
                nc.vector.bn_stats(
                    bs1[:, co, g], z1[co][:, g * FTS:(g + 1) * FTS])
        _emit_local_stats(nc, "bn1", statp, bs1, st1, agg1, P_CH)
        mv1 = _emit_exchange(nc, tc, ctx, "bn1", st1, P_CH, cc_mode)
        a1, b1 = _emit_bn_params(nc, tc, ctx, "bn1", mv1, g1, be1, P_CH, epst)

        NH = NL // 2
        for h2 in range(2):
            for c in range(P_CH):
                nc.scalar.activation(
                    y1p[c][:, h2 * NH:(h2 + 1) * NH, 1:1 + H, 1:1 + W],
                    z1[c].rearrange("p (n h w) -> p n h w", n=NL, h=H, w=W)
                        [:, h2 * NH:(h2 + 1) * NH],
                    ACTF.Relu,
                    bias=b1[:, c:c + 1],
                    scale=a1[:, c:c + 1],
                )

        # ================= stage B: conv2 (3x3, 256 -> 256) ==============
        for co in range(P_CH):
            for fp in range(2):
                pt = psum.tile([128, 2, 512], F32, name="pt", tag="pt")
                for half in range(2):
                    ft = fp * 2 + half
                    for ci in range(P_CH):
                        for tap in range(9):
                            ky, kx = divmod(tap, 3)
                            nc.tensor.matmul(
                                pt[:, half, :FTS],
                                w2sb[ci][:, tap, co * 128:(co + 1) * 128],
                                y1p[ci][:, ft * IPT:(ft + 1) * IPT,
                                        ky:ky + H, kx:kx + W],
                                start=(ci == 0 and tap == 0),
                                stop=(ci == P_CH - 1 and tap == 8),
                            )
                nc.scalar.copy(
                    z2[co][:, fp * 2 * FTS:(fp + 1) * 2 * FTS]
                        .rearrange("p (a b) -> p a b", a=2),
                    pt[:, :, :FTS],
                )
            for g in range(SG):
                nc.vector.bn_stats(
                    bs2[:, co, g], z2[co][:, g * FTS:(g + 1) * FTS])
        _emit_local_stats(nc, "bn2", statp, bs2, st2, agg2, P_CH)
        mv2 = _emit_exchange(nc, tc, ctx, "bn2", st2, P_CH, cc_mode)
        a2, b2 = _emit_bn_params(nc, tc, ctx, "bn2", mv2, g2, be2, P_CH, epst)

        for h2 in range(2):
            for c in range(P_CH):
                nc.scalar.activation(
                    y2[c][:, h2 * 2 * FTS:(h2 + 1) * 2 * FTS],
                    z2[c][:, h2 * 2 * FTS:(h2 + 1) * 2 * FTS], ACTF.Relu,
                    bias=b2[:, c:c + 1], scale=a2[:, c:c + 1],
                )

        # ================= stage C: conv3 (1x1, 256 -> 1024) =============
        for co in range(CO_CH):
            for fp in range(2):
                pt = psum.tile([128, 2, 512], F32, name="pt", tag="pt")
                for half in range(2):
                    ft = fp * 2 + half
                    for ci in range(P_CH):
                        nc.tensor.matmul(
                            pt[:, half, :FTS],
                            w3sb[ci][:, co * 128:(co + 1) * 128],
                            y2[ci][:, ft * FTS:(ft + 1) * FTS],
                            start=(ci == 0),
                            stop=(ci == P_CH - 1),
                        )
                nc.scalar.copy(
                    z3[co][:, fp * 2 * FTS:(fp + 1) * 2 * FTS]
                        .rearrange("p (a b) -> p a b", a=2),
                    pt[:, :, :FTS],
                )
            for g in range(SG):
                nc.vector.bn_stats(
                    bs3[:, co, g], z3[co][:, g * FTS:(g + 1) * FTS])
        _emit_local_stats(nc, "bn3", statp, bs3, st3, agg3, CO_CH)
        mv3 = _emit_exchange(nc, tc, ctx, "bn3", st3, CO_CH, cc_mode)
        a3, b3 = _emit_bn_params(nc, tc, ctx, "bn3", mv3, g3, be3, CO_CH, epst)

        # tail: out = relu((a3*z3 + x) + b3); t in bf16 (2x DVE mode), the
        # relu pass converts to fp32 for the output DMA.
        # tail per chunk: t = a3*z3 (TS, 4x bf16) ; t += x (TT, 2x bf16) ;
        # out = relu(t + b3) fp32 — ACT for six chunks, DVE add+max for two.
        outf = [actp.tile([128, F], F32, name=f"outf{c}") for c in range(CO_CH)]
        tts = [scrp.tile([128, F], BF16, name=f"t{c}", tag=f"t{c % 4}")
               for c in range(CO_CH)]
        for co in range(CO_CH):
            t = tts[co]
            nc.vector.tensor_scalar_mul(t[:], z3[co][:], a3[:, co:co + 1])
            nc.vector.tensor_add(t[:], t[:], xb[co][:])
            if co < 6:
                nc.scalar.activation(
                    outf[co][:], t[:], ACTF.Relu, bias=b3[:, co:co + 1],
                )
            else:
                nc.vector.tensor_scalar(
                    outf[co][:], t[:], b3[:, co:co + 1], 0.0,
                    op0=ALU.add, op1=ALU.max,
                )
            deng = nc.sync if co % 2 == 0 else nc.scalar
            deng.dma_start(
                out_d[:, co * 128:(co + 1) * 128, :].rearrange("n p f -> p n f"),
                outf[co][:],
            )
    nc.compile()
    return nc


_NC_CACHE = None


def _get_nc():
    global _NC_CACHE
    if _NC_CACHE is None:
        _NC_CACHE = build()
    return _NC_CACHE


def _prep_host(w1, w2, w3, g1, be1, g2, be2, g3, be3, residual_scale):
    bf = ml_dtypes.bfloat16
    # conv weights, pre-transposed to [ci, ...] layouts for lhsT
    w1t = np.ascontiguousarray(
        w1.reshape(P, CIN).T.astype(bf)).reshape(CI_CH, 128, P)
    w2t = np.ascontiguousarray(
        w2.transpose(1, 2, 3, 0).astype(bf)).reshape(P_CH, 128, 9, P)
    w3t = np.ascontiguousarray(
        w3.reshape(COUT, P).T.astype(bf)).reshape(P_CH, 128, COUT)
    s = np.float32(np.log1p(np.exp(np.float64(residual_scale[0]))))
    gb1 = np.ascontiguousarray(np.stack([g1, be1]).astype(np.float32)
                               .reshape(2, P_CH, 128).transpose(0, 2, 1))
    gb2 = np.ascontiguousarray(np.stack([g2, be2]).astype(np.float32)
                               .reshape(2, P_CH, 128).transpose(0, 2, 1))
    gb3 = np.ascontiguousarray((np.stack([g3, be3]) * s).astype(np.float32)
                               .reshape(2, CO_CH, 128).transpose(0, 2, 1))
    return w1t, w2t, w3t, gb1, gb2, gb3


def prepare_in_maps(inputs):
    x = np.asarray(inputs["x"], dtype=np.float32)
    w1t, w2t, w3t, gb1, gb2, gb3 = _prep_host(
        np.asarray(inputs["w1"], np.float32), np.asarray(inputs["w2"], np.float32),
        np.asarray(inputs["w3"], np.float32), np.asarray(inputs["g1"], np.float32),
        np.asarray(inputs["be1"], np.float32), np.asarray(inputs["g2"], np.float32),
        np.asarray(inputs["be2"], np.float32), np.asarray(inputs["g3"], np.float32),
        np.asarray(inputs["be3"], np.float32),
        np.asarray(inputs["residual_scale"], np.float32),
    )
    in_maps = []
    for c in range(N_CORES):
        shard = x[c * NL:(c + 1) * NL].reshape(NL, CIN, HW)
        xb16 = np.ascontiguousarray(
            shard.transpose(1, 0, 2).astype(ml_dtypes.bfloat16)
        ).reshape(CI_CH, 128, F)
        in_maps.append({
            "xb16": xb16, "w1t": w1t, "w2t": w2t, "w3t": w3t,
            "gb1": gb1, "gb2": gb2, "gb3": gb3,
        })
    return in_maps


def kernel(**inputs):
    in_maps = prepare_in_maps(inputs)
    nc = _get_nc()
    trace = bool(int(os.environ.get("KERNEL_PROFILE", "0")))
    try:
        res = run_bass_kernel_spmd(nc, in_maps, list(range(N_CORES)), trace=trace)
    except ModuleNotFoundError:
        # axon NTFF profile hook unavailable in this container
        res = run_bass_kernel_spmd(nc, in_maps, list(range(N_CORES)), trace=False)
    if trace:
        kernel.last_exec_time_ns = getattr(res, "exec_time_ns", None)
        kernel.last_profile = res
    out = np.concatenate([res.results[c]["out"] for c in range(N_CORES)], axis=0)
    return out.reshape(N, CIN, H, W)
